# revision 3
# baseline (speedup 1.0000x reference)
"""Trainium2 Bass kernel for nn_BaseMetricS2 (histogram_binning).

Math: the reference returns (mean(tp), mean(fp), mean(fn), mean(tn)) over the
(B, C) grid.  Summing the per-class identities over classes collapses the
whole problem to one weighted match-count per batch element:

    sum_c tp[b,c] = sum_px qw * [argmax_c pred == truth]      =: Wm_b
    sum_c fn[b,c] = sum_c fp[b,c] = S - Wm_b                  (S = sum qw)
    sum_c tn[b,c] = (C-2)*S + Wm_b

so no per-class histograms are needed on device.  Each of the 8 cores takes
one batch element (data-parallel over batch, per the sharding hint) and
computes unweighted per-(row-tile, partition) match counts; the host applies
the per-latitude quadrature weight (qw is constant along longitude) and the
final means.

Device pipeline per core, per [128-row x 720-col] chunk:
  1. DMA the 16 class planes into one SBUF tile [128, 16, 720].
  2. Stuff the class id into the 4 low mantissa bits of each f32 plane
     (tensor_scalar and/or, in place): v' = (v & ~0xF) | (15 - c).
  3. Grouped max-reduce over the class axis -> stuffed max per pixel.
  4. idx = (m' & 0xF) ^ 0xF  (argmax index; low-bit masking flips argmax only
     when the top-2 classes agree in their top 28 bits -- ~1e-6 of pixels,
     far below the output tolerance).
  5. tensor_tensor(is_equal(idx, truth)) -> f32 matched mask; ScalarE
     activation(Identity, accum_out) sums it per partition (TTR is broken on
     this runtime; the ACT-side sum also keeps the reduce off the busy DVE).

Row tiling: 721 rows = 5 full 128-row tiles + one final tile at rows
593..720 (overlapping rows 593..639, which the host masks out).
"""

import numpy as np

NLAT, NLON = 721, 1440
C = 16
N_CORES = 8
W_HALF = 720
TILE_R0 = (0, 128, 256, 384, 512, 593)
NCHUNK = len(TILE_R0) * 2  # 12

_CACHE = {}


def _build_program():
    from contextlib import ExitStack

    import concourse.bacc as bacc
    import concourse.tile as tile
    from concourse import mybir

    F32 = mybir.dt.float32
    I32 = mybir.dt.int32
    Alu = mybir.AluOpType

    nc = bacc.Bacc("TRN2", target_bir_lowering=False, debug=False)
    pred = nc.dram_tensor("pred", [C, NLAT, NLON], F32, kind="ExternalInput").ap()
    truth = nc.dram_tensor("truth", [NLAT, NLON], I32, kind="ExternalInput").ap()
    out = nc.dram_tensor("out", [128, NCHUNK], F32, kind="ExternalOutput").ap()

    with tile.TileContext(nc) as tc, ExitStack() as ctx:
        pred_pool = ctx.enter_context(tc.tile_pool(name="pred", bufs=3))
        tr_pool = ctx.enter_context(tc.tile_pool(name="tr", bufs=2))
        m_pool = ctx.enter_context(tc.tile_pool(name="m", bufs=2))
        idx_pool = ctx.enter_context(tc.tile_pool(name="idx", bufs=2))
        scr_pool = ctx.enter_context(tc.tile_pool(name="scr", bufs=2))
        acc_pool = ctx.enter_context(tc.tile_pool(name="acc", bufs=1))

        acc = acc_pool.tile([128, NCHUNK], F32)

        for t, r0 in enumerate(TILE_R0):
            for h in range(2):
                w0 = h * W_HALF
                k = t * 2 + h

                pt = pred_pool.tile([128, C, W_HALF], F32, tag="pred")
                nc.sync.dma_start(
                    pt[:, :, :],
                    pred[:, r0 : r0 + 128, w0 : w0 + W_HALF].rearrange(
                        "c r w -> r c w"
                    ),
                )
                tt = tr_pool.tile([128, W_HALF], I32, tag="tr")
                nc.sync.dma_start(tt[:, :], truth[r0 : r0 + 128, w0 : w0 + W_HALF])

                for c in range(C):
                    sl = pt[:, c, :].bitcast(I32)
                    nc.vector.tensor_scalar(
                        sl, sl, -16, 15 - c, op0=Alu.bitwise_and, op1=Alu.bitwise_or
                    )

                mt = m_pool.tile([128, W_HALF], F32, tag="m")
                nc.vector.tensor_reduce(
                    mt[:, :],
                    pt[:, :, :].rearrange("p c w -> p w c"),
                    axis=mybir.AxisListType.X,
                    op=Alu.max,
                )

                it = idx_pool.tile([128, W_HALF], I32, tag="idx")
                nc.vector.tensor_scalar(
                    it[:, :], mt[:, :].bitcast(I32), 15, 15,
                    op0=Alu.bitwise_and, op1=Alu.bitwise_xor,
                )

                st = scr_pool.tile([128, W_HALF], F32, tag="scr")
                nc.vector.tensor_tensor(
                    st[:, :], it[:, :], tt[:, :], op=Alu.is_equal
                )
                nc.scalar.activation(
                    st[:, :], st[:, :], mybir.ActivationFunctionType.Identity,
                    accum_out=acc[:, k : k + 1],
                )

        nc.sync.dma_start(out[:, :], acc[:, :])

    nc.compile()
    return nc


def _get_program():
    if "nc" not in _CACHE:
        _CACHE["nc"] = _build_program()
    return _CACHE["nc"]


def kernel(pred: np.ndarray, truth: np.ndarray, quad_weights: np.ndarray):
    from concourse.bass_utils import run_bass_kernel_spmd

    assert pred.shape == (N_CORES, C, NLAT, NLON), pred.shape
    pred = np.ascontiguousarray(pred, dtype=np.float32)
    truth_i32 = np.ascontiguousarray(truth.astype(np.int32))

    nc = _get_program()
    in_maps = [
        {"pred": pred[b], "truth": truth_i32[b]} for b in range(N_CORES)
    ]
    results = run_bass_kernel_spmd(nc, in_maps, list(range(N_CORES))).results

    # Host reduction: apply per-latitude quadrature weights and the means.
    qw = np.asarray(quad_weights, dtype=np.float64)
    w_row = qw[:, 0]  # qw is constant along longitude by construction
    S = float(qw.sum())

    wm = np.zeros(N_CORES, dtype=np.float64)
    for b in range(N_CORES):
        counts = np.asarray(results[b]["out"], dtype=np.float64)  # [128, 12]
        for t, r0 in enumerate(TILE_R0):
            per_row = counts[:, 2 * t] + counts[:, 2 * t + 1]  # [128]
            rows = r0 + np.arange(128)
            if t == len(TILE_R0) - 1:
                per_row = per_row[47:]  # rows 593..639 already counted in tile 4
                rows = rows[47:]
            wm[b] += float(np.dot(w_row[rows], per_row))

    denom = N_CORES * C
    tp_mean = wm.sum() / denom
    fp_mean = (N_CORES * S - wm.sum()) / denom
    fn_mean = fp_mean
    tn_mean = ((C - 2) * S * N_CORES + wm.sum()) / denom
    return (
        np.float32(tp_mean),
        np.float32(fp_mean),
        np.float32(fn_mean),
        np.float32(tn_mean),
    )


# revision 4
# speedup vs baseline: 15.0227x; 15.0227x over previous
"""Trainium2 Bass kernel for nn_BaseMetricS2 (histogram_binning).

Math: the reference returns (mean(tp), mean(fp), mean(fn), mean(tn)) over the
(B, C) grid.  Summing the per-class identities over classes collapses the
whole problem to one weighted match-count per batch element:

    sum_c tp[b,c] = sum_px qw * [argmax_c pred == truth]      =: Wm_b
    sum_c fn[b,c] = sum_c fp[b,c] = S - Wm_b                  (S = sum qw)
    sum_c tn[b,c] = (C-2)*S + Wm_b

so no per-class histograms are needed on device.  Each of the 8 cores takes
one batch element (data-parallel over batch, per the sharding hint) and
computes unweighted per-(row-tile, partition) match counts; the host applies
the per-latitude quadrature weight (qw is constant along longitude) and the
final means.

Device pipeline per core, per [128-row x 720-col] chunk:
  1. DMA the 16 class planes into one SBUF tile [128, 16, 720].
  2. Stuff the class id into the 4 low mantissa bits of each f32 plane
     (tensor_scalar and/or, in place): v' = (v & ~0xF) | (15 - c).
  3. Grouped max-reduce over the class axis -> stuffed max per pixel.
  4. idx = (m' & 0xF) ^ 0xF  (argmax index; low-bit masking flips argmax only
     when the top-2 classes agree in their top 28 bits -- ~1e-6 of pixels,
     far below the output tolerance).
  5. tensor_tensor(is_equal(idx, truth)) -> f32 matched mask; ScalarE
     activation(Identity, accum_out) sums it per partition (TTR is broken on
     this runtime; the ACT-side sum also keeps the reduce off the busy DVE).

Row tiling: 721 rows = 5 full 128-row tiles + one final tile at rows
593..720 (overlapping rows 593..639, which the host masks out).
"""

import numpy as np

NLAT, NLON = 721, 1440
C = 16
N_CORES = 8
W_HALF = 720
TILE_R0 = (0, 128, 256, 384, 512, 593)
NCHUNK = len(TILE_R0) * 2  # 12

_CACHE = {}


def _build_program(repeat=1, pred_bufs=3):
    """Build the Bass program.  repeat>1 replays the whole body (same data)
    for slope-based wall-clock timing; the graded path uses repeat=1."""
    from contextlib import ExitStack

    import concourse.bacc as bacc
    import concourse.tile as tile
    from concourse import mybir

    F32 = mybir.dt.float32
    I32 = mybir.dt.int32
    Alu = mybir.AluOpType

    nc = bacc.Bacc("TRN2", target_bir_lowering=False, debug=False)
    pred = nc.dram_tensor("pred", [C, NLAT, NLON], F32, kind="ExternalInput").ap()
    truth = nc.dram_tensor("truth", [NLAT, NLON], I32, kind="ExternalInput").ap()
    out = nc.dram_tensor("out", [128, NCHUNK], F32, kind="ExternalOutput").ap()

    with tile.TileContext(nc) as tc, ExitStack() as ctx:
        pred_pool = ctx.enter_context(tc.tile_pool(name="pred", bufs=pred_bufs))
        tr_pool = ctx.enter_context(tc.tile_pool(name="tr", bufs=2))
        m_pool = ctx.enter_context(tc.tile_pool(name="m", bufs=2))
        idx_pool = ctx.enter_context(tc.tile_pool(name="idx", bufs=2))
        scr_pool = ctx.enter_context(tc.tile_pool(name="scr", bufs=2))
        acc_pool = ctx.enter_context(tc.tile_pool(name="acc", bufs=1))

        acc = acc_pool.tile([128, NCHUNK], F32)

        for _rep in range(repeat):
            for t, r0 in enumerate(TILE_R0):
                for h in range(2):
                    w0 = h * W_HALF
                    k = t * 2 + h

                    pt = pred_pool.tile([128, C, W_HALF], F32, tag="pred")
                    nc.sync.dma_start(
                        pt[:, :, :],
                        pred[:, r0 : r0 + 128, w0 : w0 + W_HALF].rearrange(
                            "c r w -> r c w"
                        ),
                    )
                    tt = tr_pool.tile([128, W_HALF], I32, tag="tr")
                    nc.sync.dma_start(tt[:, :], truth[r0 : r0 + 128, w0 : w0 + W_HALF])

                    for c in range(C):
                        sl = pt[:, c, :].bitcast(I32)
                        nc.vector.tensor_scalar(
                            sl, sl, -16, 15 - c, op0=Alu.bitwise_and, op1=Alu.bitwise_or
                        )

                    mt = m_pool.tile([128, W_HALF], F32, tag="m")
                    nc.vector.tensor_reduce(
                        mt[:, :],
                        pt[:, :, :].rearrange("p c w -> p w c"),
                        axis=mybir.AxisListType.X,
                        op=Alu.max,
                    )

                    it = idx_pool.tile([128, W_HALF], I32, tag="idx")
                    nc.vector.tensor_scalar(
                        it[:, :], mt[:, :].bitcast(I32), 15, 15,
                        op0=Alu.bitwise_and, op1=Alu.bitwise_xor,
                    )

                    st = scr_pool.tile([128, W_HALF], F32, tag="scr")
                    nc.vector.tensor_tensor(
                        st[:, :], it[:, :], tt[:, :], op=Alu.is_equal
                    )
                    nc.scalar.activation(
                        st[:, :], st[:, :], mybir.ActivationFunctionType.Identity,
                        accum_out=acc[:, k : k + 1],
                    )

        nc.sync.dma_start(out[:, :], acc[:, :])

    nc.compile()
    return nc


def _get_program():
    if "nc" not in _CACHE:
        _CACHE["nc"] = _build_program()
    return _CACHE["nc"]


def kernel(pred: np.ndarray, truth: np.ndarray, quad_weights: np.ndarray):
    from concourse.bass_utils import run_bass_kernel_spmd

    assert pred.shape == (N_CORES, C, NLAT, NLON), pred.shape
    pred = np.ascontiguousarray(pred, dtype=np.float32)
    truth_i32 = np.ascontiguousarray(truth.astype(np.int32))

    nc = _get_program()
    in_maps = [
        {"pred": pred[b], "truth": truth_i32[b]} for b in range(N_CORES)
    ]
    results = run_bass_kernel_spmd(nc, in_maps, list(range(N_CORES))).results

    # Host reduction: apply per-latitude quadrature weights and the means.
    qw = np.asarray(quad_weights, dtype=np.float64)
    w_row = qw[:, 0]  # qw is constant along longitude by construction
    S = float(qw.sum())

    wm = np.zeros(N_CORES, dtype=np.float64)
    for b in range(N_CORES):
        counts = np.asarray(results[b]["out"], dtype=np.float64)  # [128, 12]
        for t, r0 in enumerate(TILE_R0):
            per_row = counts[:, 2 * t] + counts[:, 2 * t + 1]  # [128]
            rows = r0 + np.arange(128)
            if t == len(TILE_R0) - 1:
                per_row = per_row[47:]  # rows 593..639 already counted in tile 4
                rows = rows[47:]
            wm[b] += float(np.dot(w_row[rows], per_row))

    denom = N_CORES * C
    tp_mean = wm.sum() / denom
    fp_mean = (N_CORES * S - wm.sum()) / denom
    fn_mean = fp_mean
    tn_mean = ((C - 2) * S * N_CORES + wm.sum()) / denom
    return (
        np.float32(tp_mean),
        np.float32(fp_mean),
        np.float32(fn_mean),
        np.float32(tn_mean),
    )
